# revision 7
# baseline (speedup 1.0000x reference)
"""BayesianLinear TRN2 kernel: out = x @ (mu + (softplus(rho)+1e-8)*eps).T + bias.

Full shapes: x [4096, 4096], weight_* [4096(out), 4096(in)], bias_* [4096].
Sharding across 8 NeuronCores: 2 batch-halves x 4 out-groups.
  core c: batch rows [ (c//4)*2048 : ... ), out cols [ (c%4)*1024 : ... ).
Per core the kernel computes the TRANSPOSED shard outT [1024(out), 2048(batch)]
= W_g @ x_h.T; the host assemble() transposes back (layout-only).

v2 design (vs v1 which was x-stationary f32r):
- The sampled weight W = mu + softplus(rho)*eps is the matmul STATIONARY
  operand in BF16 — it is produced directly by the DVE add (output cast),
  so no extra conversion pass. bf16 stationary gets the fast weight-load
  path; f32r stationary self-loads 4B columns serially per matmul and was
  the main model-vs-hardware gap in v1.
- x streams as the MOVING operand in f32r via AP bitcast of the f32 DMA
  bytes: no gpsimd conversion pass at all. Moving f32r at free-size 512
  runs 1 row/cycle (full rate).
- Output tiles are [128 out, 512 batch] in PSUM (1 bank), accumulated over
  4 K-chunks per phase, 8 phases; SBUF fp32 accumulator across phases.
- Bias is per-partition in this layout: one tensor_scalar add with a
  [128,1] scalar AP, fused into the phase-0 accumulator write.

Per-core roofline: DMA 92 MB / 360 GBps = 256 us; PE 1024 matmuls x 512
rows x 0.4167 ns = 218 us. Target ~260-280 us.
"""
import numpy as np
from contextlib import ExitStack

import concourse.tile as tile
import concourse.mybir as mybir
from concourse import bacc

P = 128
IN_F = 4096           # contraction (in_features)
BATCH = 4096
OUT_F = 4096
B_CORE = 2048         # batch cols per core (2 halves)
O_CORE = 1024         # out rows per core (4 groups)
N_KC = IN_F // P      # 32 k-chunks of 128
N_PHASES = 8
KC_P = N_KC // N_PHASES   # 4 k-chunks per phase
OT = O_CORE // P      # 8 out-tiles of 128
BG = B_CORE // 512    # 4 batch-groups of 512

F32 = mybir.dt.float32
F32R = mybir.dt.float32r
BF16 = mybir.dt.bfloat16
ACT = mybir.ActivationFunctionType
ALU = mybir.AluOpType

_CACHE = {}


def build_nc(inner_reps=1):
    nc = bacc.Bacc("TRN2", debug=False, num_devices=8)
    xt = nc.dram_tensor("xt", (IN_F, B_CORE), F32, kind="ExternalInput").ap()
    wtm = nc.dram_tensor("wtm", (IN_F, O_CORE), F32, kind="ExternalInput").ap()
    wtr = nc.dram_tensor("wtr", (IN_F, O_CORE), F32, kind="ExternalInput").ap()
    wte = nc.dram_tensor("wte", (IN_F, O_CORE), F32, kind="ExternalInput").ap()
    bm = nc.dram_tensor("bm", (O_CORE,), F32, kind="ExternalInput").ap()
    br = nc.dram_tensor("br", (O_CORE,), F32, kind="ExternalInput").ap()
    be = nc.dram_tensor("be", (O_CORE,), F32, kind="ExternalInput").ap()
    out = nc.dram_tensor("out", (O_CORE, B_CORE), F32, kind="ExternalOutput").ap()

    xt_r = xt.rearrange("(kc p) b -> p kc b", p=P)       # [128, 32, 2048]
    wm_r = wtm.rearrange("(kc p) o -> kc p o", p=P)      # [32, 128, 1024]
    wr_r = wtr.rearrange("(kc p) o -> kc p o", p=P)
    we_r = wte.rearrange("(kc p) o -> kc p o", p=P)
    out_r = out.rearrange("(ot p) b -> ot p b", p=P)     # [8, 128, 2048]
    bm_r = bm.rearrange("(ot p) -> p ot", p=P)           # [128, 8]
    br_r = br.rearrange("(ot p) -> p ot", p=P)
    be_r = be.rearrange("(ot p) -> p ot", p=P)

    with ExitStack() as ctx:
        tc = ctx.enter_context(tile.TileContext(nc))
        wstage = ctx.enter_context(tc.tile_pool(name="ws", bufs=3))
        wpool = ctx.enter_context(tc.tile_pool(name="w", bufs=2))
        xpool = ctx.enter_context(tc.tile_pool(name="x", bufs=2))
        accpool = ctx.enter_context(tc.tile_pool(name="acc", bufs=1))
        bpool = ctx.enter_context(tc.tile_pool(name="bias", bufs=1))
        pspool = ctx.enter_context(tc.tile_pool(name="ps", bufs=2, space="PSUM"))

        acc = accpool.tile([P, OT, B_CORE], F32)        # 64KB/partition
        bias_t = bpool.tile([P, OT], F32, tag="bias")

        def prep_bias():
            tb_r = bpool.tile([P, OT], F32, tag="b_r")
            tb_m = bpool.tile([P, OT], F32, tag="b_m")
            tb_e = bpool.tile([P, OT], F32, tag="b_e")
            nc.scalar.dma_start(tb_r[:], br_r)
            nc.scalar.dma_start(tb_m[:], bm_r)
            nc.scalar.dma_start(tb_e[:], be_r)
            nc.scalar.activation(tb_r[:], tb_r[:], ACT.Exp)
            nc.scalar.activation(tb_r[:], tb_r[:], ACT.Ln, bias=1.0)
            nc.vector.scalar_tensor_tensor(tb_r[:], tb_r[:], 1e-8, tb_e[:],
                                           ALU.add, ALU.mult)
            nc.vector.tensor_add(bias_t[:], tb_r[:], tb_m[:])

        def prep_phase(p):
            kc0 = KC_P * p
            w_p = wpool.tile([P, KC_P, O_CORE], BF16, tag="w", name="w_p")
            for kc in range(KC_P):
                k = kc0 + kc
                t_r = wstage.tile([P, O_CORE], F32, tag="rho")
                t_m = wstage.tile([P, O_CORE], F32, tag="mu")
                t_e = wstage.tile([P, O_CORE], F32, tag="eps")
                nc.sync.dma_start(t_r[:], wr_r[k])
                nc.sync.dma_start(t_m[:], wm_r[k])
                nc.sync.dma_start(t_e[:], we_r[k])
                nc.scalar.activation(t_r[:], t_r[:], ACT.Exp)
                nc.scalar.activation(t_r[:], t_r[:], ACT.Ln, bias=1.0)
                # (sigma+1e-8)*eps; in-place into t_r
                nc.vector.scalar_tensor_tensor(t_r[:], t_r[:], 1e-8, t_e[:],
                                               ALU.add, ALU.mult)
                nc.vector.tensor_add(w_p[:, kc], t_r[:], t_m[:])
            return w_p

        for _rep in range(inner_reps):
            if _rep == 0:
                prep_bias()
            for p in range(N_PHASES):
                kc0 = KC_P * p
                w_p = prep_phase(p)
                # x streamed in as bf16 via casting SWDGE DMA on the (otherwise
                # idle) Pool engine queue; split per 512-batch chunk so the
                # first matmuls ungate after 1/4 of the phase's x bytes.
                xs = xpool.tile([P, KC_P, B_CORE], BF16, tag="xs", name="xs")
                for bg in range(BG):
                    nc.gpsimd.dma_start(
                        xs[:, :, bg * 512:(bg + 1) * 512],
                        xt_r[:, kc0:kc0 + KC_P, bg * 512:(bg + 1) * 512])
                for ot in range(OT):
                    ps = pspool.tile([P, B_CORE], F32, tag="ps")
                    for kc in range(KC_P):
                        for bg in range(BG):
                            nc.tensor.matmul(
                                ps[:, bg * 512:(bg + 1) * 512],
                                w_p[:, kc, ot * P:(ot + 1) * P],
                                xs[:, kc, bg * 512:(bg + 1) * 512],
                                start=(kc == 0),
                                stop=(kc == KC_P - 1),
                            )
                    a = acc[:, ot, :]
                    if p == 0:
                        nc.vector.tensor_scalar(
                            a, ps[:], bias_t[:, ot:ot + 1], None, ALU.add)
                    else:
                        nc.vector.tensor_add(a, a, ps[:])
                    if p == N_PHASES - 1:
                        nc.scalar.dma_start(out_r[ot], a)
    nc.compile()
    return nc


# ---------------------------------------------------------------------------
# host-side runner (PJRT under axon)
# ---------------------------------------------------------------------------

def _prepare_fn(nc, n_cores=8):
    import jax
    from jax.sharding import Mesh, PartitionSpec
    from jax.experimental.shard_map import shard_map
    from concourse.bass2jax import (
        _bass_exec_p, install_neuronx_cc_hook, partition_id_tensor,
    )

    install_neuronx_cc_hook()
    pname = nc.partition_id_tensor.name if nc.partition_id_tensor else None
    in_names, out_names, out_avals = [], [], []
    for alloc in nc.m.functions[0].allocations:
        if not isinstance(alloc, mybir.MemoryLocationSet):
            continue
        name = alloc.memorylocations[0].name
        if alloc.kind == "ExternalInput":
            if name != pname:
                in_names.append(name)
        elif alloc.kind == "ExternalOutput":
            out_names.append(name)
            out_avals.append(
                jax.core.ShapedArray(tuple(alloc.tensor_shape), mybir.dt.np(alloc.dtype))
            )

    all_in = list(in_names) + list(out_names) + ([pname] if pname else [])

    def _body(*args):
        ops = list(args)
        if pname:
            ops.append(partition_id_tensor())
        return tuple(
            _bass_exec_p.bind(
                *ops,
                out_avals=tuple(out_avals),
                in_names=tuple(all_in),
                out_names=tuple(out_names),
                lowering_input_output_aliases=(),
                sim_require_finite=True,
                sim_require_nnan=True,
                nc=nc,
            )
        )

    devices = jax.devices()[:n_cores]
    mesh = Mesh(np.asarray(devices), ("core",))
    nargs = len(in_names) + len(out_names)
    fn = jax.jit(
        shard_map(
            _body, mesh=mesh,
            in_specs=(PartitionSpec("core"),) * nargs,
            out_specs=(PartitionSpec("core"),) * len(out_names),
            check_rep=False,
        ),
        keep_unused=True,
    )
    return fn, mesh, in_names, out_names, out_avals


def get_compiled(inner_reps=1):
    key = ("fn", inner_reps)
    if key not in _CACHE:
        nc = build_nc(inner_reps)
        _CACHE[key] = _prepare_fn(nc)
    return _CACHE[key]


def shard_inputs(x, weight_mu, weight_rho, bias_mu, bias_rho, weight_eps, bias_eps):
    """Returns in_maps (list of dicts, one per core). Layout-only transforms."""
    xT = np.ascontiguousarray(np.asarray(x).T)          # [in, batch]
    in_maps = []
    for c in range(8):
        h, g = divmod(c, 4)
        o0 = g * O_CORE
        in_maps.append({
            "xt": np.ascontiguousarray(xT[:, h * B_CORE:(h + 1) * B_CORE]),
            "wtm": np.ascontiguousarray(np.asarray(weight_mu)[o0:o0 + O_CORE, :].T),
            "wtr": np.ascontiguousarray(np.asarray(weight_rho)[o0:o0 + O_CORE, :].T),
            "wte": np.ascontiguousarray(np.asarray(weight_eps)[o0:o0 + O_CORE, :].T),
            "bm": np.asarray(bias_mu)[o0:o0 + O_CORE],
            "br": np.asarray(bias_rho)[o0:o0 + O_CORE],
            "be": np.asarray(bias_eps)[o0:o0 + O_CORE],
        })
    return in_maps


def run_device(in_maps, inner_reps=1):
    import jax
    from jax.sharding import NamedSharding, PartitionSpec

    fn, mesh, in_names, out_names, out_avals = get_compiled(inner_reps)
    sh = NamedSharding(mesh, PartitionSpec("core"))
    concat_in = [
        np.concatenate([np.asarray(in_maps[c][nm]) for c in range(8)], axis=0)
        for nm in in_names
    ]
    dev_in = [jax.device_put(a, sh) for a in concat_in]
    dev_z = [
        jax.device_put(np.zeros((8 * a.shape[0], *a.shape[1:]), a.dtype), sh)
        for a in out_avals
    ]
    out_arrs = fn(*dev_in, *dev_z)
    jax.block_until_ready(out_arrs)
    i_out = out_names.index("out")
    outs = np.asarray(out_arrs[i_out]).reshape(8, O_CORE, B_CORE)
    return outs, (fn, dev_in, dev_z)


def assemble(outs):
    full = np.empty((BATCH, OUT_F), dtype=np.float32)
    for c in range(8):
        h, g = divmod(c, 4)
        full[h * B_CORE:(h + 1) * B_CORE, g * O_CORE:(g + 1) * O_CORE] = outs[c].T
    return full


def kernel(**inputs) -> np.ndarray:
    in_maps = shard_inputs(**inputs)
    outs, _ = run_device(in_maps)
    return assemble(outs)


if __name__ == "__main__":
    rng = np.random.default_rng(0)
    ins = {
        "x": rng.standard_normal((BATCH, IN_F), dtype=np.float32),
        "weight_mu": (rng.standard_normal((OUT_F, IN_F), dtype=np.float32)
                      * np.sqrt(2.0 / IN_F)).astype(np.float32),
        "weight_rho": rng.uniform(-5.5, -2.5, (OUT_F, IN_F)).astype(np.float32),
        "bias_mu": np.zeros(OUT_F, dtype=np.float32),
        "bias_rho": rng.uniform(-5.5, -2.5, OUT_F).astype(np.float32),
        "weight_eps": rng.standard_normal((OUT_F, IN_F), dtype=np.float32),
        "bias_eps": rng.standard_normal(OUT_F, dtype=np.float32),
    }
    got = kernel(**ins)
    w = ins["weight_mu"] + (np.log1p(np.exp(ins["weight_rho"].astype(np.float64))) + 1e-8) * ins["weight_eps"]
    b = ins["bias_mu"] + (np.log1p(np.exp(ins["bias_rho"].astype(np.float64))) + 1e-8) * ins["bias_eps"]
    ref = ins["x"].astype(np.float64) @ w.T + b
    rel = np.linalg.norm(got - ref) / np.linalg.norm(ref)
    print("L2 rel err vs fp64 numpy:", rel)


# revision 12
# speedup vs baseline: 2.4081x; 2.4081x over previous
"""BayesianLinear TRN2 kernel: out = x @ (mu + (softplus(rho)+1e-8)*eps).T + bias.

Full shapes: x [4096, 4096], weight_* [4096(out), 4096(in)], bias_* [4096].
Sharding across 8 NeuronCores: 2 batch-halves x 4 out-groups.
  core c: batch rows [ (c//4)*2048 : ... ), out cols [ (c%4)*1024 : ... ).
Per core the kernel computes the TRANSPOSED shard outT [1024(out), 2048(batch)]
= W_g @ x_h.T; the host assemble() transposes back (layout-only).

v2 design (vs v1 which was x-stationary f32r):
- The sampled weight W = mu + softplus(rho)*eps is the matmul STATIONARY
  operand in BF16 — it is produced directly by the DVE add (output cast),
  so no extra conversion pass. bf16 stationary gets the fast weight-load
  path; f32r stationary self-loads 4B columns serially per matmul and was
  the main model-vs-hardware gap in v1.
- x streams as the MOVING operand in f32r via AP bitcast of the f32 DMA
  bytes: no gpsimd conversion pass at all. Moving f32r at free-size 512
  runs 1 row/cycle (full rate).
- Output tiles are [128 out, 512 batch] in PSUM (1 bank), accumulated over
  4 K-chunks per phase, 8 phases; SBUF fp32 accumulator across phases.
- Bias is per-partition in this layout: one tensor_scalar add with a
  [128,1] scalar AP, fused into the phase-0 accumulator write.

Per-core roofline: DMA 92 MB / 360 GBps = 256 us; PE 1024 matmuls x 512
rows x 0.4167 ns = 218 us. Target ~260-280 us.
"""
import numpy as np
from contextlib import ExitStack

import concourse.tile as tile
import concourse.mybir as mybir
from concourse import bacc

P = 128
IN_F = 4096           # contraction (in_features)
BATCH = 4096
OUT_F = 4096
B_CORE = 2048         # batch cols per core (2 halves)
O_CORE = 1024         # out rows per core (4 groups)
N_KC = IN_F // P      # 32 k-chunks of 128
N_PHASES = 8
KC_P = N_KC // N_PHASES   # 4 k-chunks per phase
OT = O_CORE // P      # 8 out-tiles of 128
BG = B_CORE // 512    # 4 batch-groups of 512

F32 = mybir.dt.float32
F32R = mybir.dt.float32r
BF16 = mybir.dt.bfloat16
ACT = mybir.ActivationFunctionType
ALU = mybir.AluOpType

_CACHE = {}


def build_nc(inner_reps=1):
    nc = bacc.Bacc("TRN2", debug=False, num_devices=8)
    xt = nc.dram_tensor("xt", (IN_F, B_CORE), F32, kind="ExternalInput").ap()
    wtm = nc.dram_tensor("wtm", (IN_F, O_CORE), F32, kind="ExternalInput").ap()
    wtr = nc.dram_tensor("wtr", (IN_F, O_CORE), F32, kind="ExternalInput").ap()
    wte = nc.dram_tensor("wte", (IN_F, O_CORE), F32, kind="ExternalInput").ap()
    bm = nc.dram_tensor("bm", (O_CORE,), F32, kind="ExternalInput").ap()
    br = nc.dram_tensor("br", (O_CORE,), F32, kind="ExternalInput").ap()
    be = nc.dram_tensor("be", (O_CORE,), F32, kind="ExternalInput").ap()
    out = nc.dram_tensor("out", (O_CORE, B_CORE), F32, kind="ExternalOutput").ap()

    xt_r = xt.rearrange("(kc p) b -> p kc b", p=P)       # [128, 32, 2048]
    wm_r = wtm.rearrange("(kc p) o -> kc p o", p=P)      # [32, 128, 1024]
    wr_r = wtr.rearrange("(kc p) o -> kc p o", p=P)
    we_r = wte.rearrange("(kc p) o -> kc p o", p=P)
    out_r = out.rearrange("(ot p) b -> ot p b", p=P)     # [8, 128, 2048]
    bm_r = bm.rearrange("(ot p) -> p ot", p=P)           # [128, 8]
    br_r = br.rearrange("(ot p) -> p ot", p=P)
    be_r = be.rearrange("(ot p) -> p ot", p=P)

    with ExitStack() as ctx:
        tc = ctx.enter_context(tile.TileContext(nc))
        wstage = ctx.enter_context(tc.tile_pool(name="ws", bufs=3))
        wpool = ctx.enter_context(tc.tile_pool(name="w", bufs=2))
        xpool = ctx.enter_context(tc.tile_pool(name="x", bufs=2))
        accpool = ctx.enter_context(tc.tile_pool(name="acc", bufs=1))
        bpool = ctx.enter_context(tc.tile_pool(name="bias", bufs=1))
        pspool = ctx.enter_context(tc.tile_pool(name="ps", bufs=2, space="PSUM"))

        acc = accpool.tile([P, OT, B_CORE], F32)        # 64KB/partition
        bias_t = bpool.tile([P, OT], F32, tag="bias")

        def prep_bias():
            tb_r = bpool.tile([P, OT], F32, tag="b_r")
            tb_m = bpool.tile([P, OT], F32, tag="b_m")
            tb_e = bpool.tile([P, OT], F32, tag="b_e")
            nc.scalar.dma_start(tb_r[:], br_r)
            nc.scalar.dma_start(tb_m[:], bm_r)
            nc.scalar.dma_start(tb_e[:], be_r)
            nc.scalar.activation(tb_r[:], tb_r[:], ACT.Exp)
            nc.scalar.activation(tb_r[:], tb_r[:], ACT.Ln, bias=1.0)
            nc.vector.scalar_tensor_tensor(tb_r[:], tb_r[:], 1e-8, tb_e[:],
                                           ALU.add, ALU.mult)
            nc.vector.tensor_add(bias_t[:], tb_r[:], tb_m[:])

        def prep_phase(p):
            kc0 = KC_P * p
            w_p = wpool.tile([P, KC_P, O_CORE], BF16, tag="w", name="w_p")
            for kc in range(KC_P):
                k = kc0 + kc
                t_r = wstage.tile([P, O_CORE], F32, tag="rho")
                t_m = wstage.tile([P, O_CORE], F32, tag="mu")
                t_e = wstage.tile([P, O_CORE], F32, tag="eps")
                nc.sync.dma_start(t_r[:], wr_r[k])
                nc.sync.dma_start(t_m[:], wm_r[k])
                nc.sync.dma_start(t_e[:], we_r[k])
                nc.scalar.activation(t_r[:], t_r[:], ACT.Exp)
                nc.scalar.activation(t_r[:], t_r[:], ACT.Ln, bias=1.0)
                # (sigma+1e-8)*eps; in-place into t_r
                nc.vector.scalar_tensor_tensor(t_r[:], t_r[:], 1e-8, t_e[:],
                                               ALU.add, ALU.mult)
                nc.vector.tensor_add(w_p[:, kc], t_r[:], t_m[:])
            return w_p

        for _rep in range(inner_reps):
            if _rep == 0:
                prep_bias()
            for p in range(N_PHASES):
                kc0 = KC_P * p
                w_p = prep_phase(p)
                # x: HWDGE f32 loads chunked per 512-batch group, cast to bf16
                # on the otherwise-idle Pool engine. (A casting SWDGE DMA would
                # skip the copy, but software descriptor prep is slow on HW.)
                xs = xpool.tile([P, KC_P, B_CORE], BF16, tag="xs", name="xs",
                                bufs=3)
                for bg in range(BG):
                    sl = slice(bg * 512, (bg + 1) * 512)
                    xs32 = xpool.tile([P, KC_P, 512], F32, tag="xs32",
                                      name="xs32", bufs=5)
                    nc.scalar.dma_start(xs32[:],
                                        xt_r[:, kc0:kc0 + KC_P, sl])
                    nc.gpsimd.tensor_copy(xs[:, :, sl], xs32[:])
                for ot in range(OT):
                    ps = pspool.tile([P, B_CORE], F32, tag="ps")
                    for kc in range(KC_P):
                        for bg in range(BG):
                            nc.tensor.matmul(
                                ps[:, bg * 512:(bg + 1) * 512],
                                w_p[:, kc, ot * P:(ot + 1) * P],
                                xs[:, kc, bg * 512:(bg + 1) * 512],
                                start=(kc == 0),
                                stop=(kc == KC_P - 1),
                            )
                    a = acc[:, ot, :]
                    if p == 0:
                        nc.vector.tensor_scalar(
                            a, ps[:], bias_t[:, ot:ot + 1], None, ALU.add)
                    else:
                        nc.vector.tensor_add(a, a, ps[:])
                    if p == N_PHASES - 1:
                        nc.scalar.dma_start(out_r[ot], a)
    nc.compile()
    return nc


# ---------------------------------------------------------------------------
# host-side runner (PJRT under axon)
# ---------------------------------------------------------------------------

def _prepare_fn(nc, n_cores=8):
    import jax
    from jax.sharding import Mesh, PartitionSpec
    from jax.experimental.shard_map import shard_map
    from concourse.bass2jax import (
        _bass_exec_p, install_neuronx_cc_hook, partition_id_tensor,
    )

    install_neuronx_cc_hook()
    pname = nc.partition_id_tensor.name if nc.partition_id_tensor else None
    in_names, out_names, out_avals = [], [], []
    for alloc in nc.m.functions[0].allocations:
        if not isinstance(alloc, mybir.MemoryLocationSet):
            continue
        name = alloc.memorylocations[0].name
        if alloc.kind == "ExternalInput":
            if name != pname:
                in_names.append(name)
        elif alloc.kind == "ExternalOutput":
            out_names.append(name)
            out_avals.append(
                jax.core.ShapedArray(tuple(alloc.tensor_shape), mybir.dt.np(alloc.dtype))
            )

    all_in = list(in_names) + list(out_names) + ([pname] if pname else [])

    def _body(*args):
        ops = list(args)
        if pname:
            ops.append(partition_id_tensor())
        return tuple(
            _bass_exec_p.bind(
                *ops,
                out_avals=tuple(out_avals),
                in_names=tuple(all_in),
                out_names=tuple(out_names),
                lowering_input_output_aliases=(),
                sim_require_finite=True,
                sim_require_nnan=True,
                nc=nc,
            )
        )

    devices = jax.devices()[:n_cores]
    mesh = Mesh(np.asarray(devices), ("core",))
    nargs = len(in_names) + len(out_names)
    fn = jax.jit(
        shard_map(
            _body, mesh=mesh,
            in_specs=(PartitionSpec("core"),) * nargs,
            out_specs=(PartitionSpec("core"),) * len(out_names),
            check_rep=False,
        ),
        keep_unused=True,
    )
    return fn, mesh, in_names, out_names, out_avals


def get_compiled(inner_reps=1):
    key = ("fn", inner_reps)
    if key not in _CACHE:
        nc = build_nc(inner_reps)
        _CACHE[key] = _prepare_fn(nc)
    return _CACHE[key]


def shard_inputs(x, weight_mu, weight_rho, bias_mu, bias_rho, weight_eps, bias_eps):
    """Returns in_maps (list of dicts, one per core). Layout-only transforms."""
    xT = np.ascontiguousarray(np.asarray(x).T)          # [in, batch]
    in_maps = []
    for c in range(8):
        h, g = divmod(c, 4)
        o0 = g * O_CORE
        in_maps.append({
            "xt": np.ascontiguousarray(xT[:, h * B_CORE:(h + 1) * B_CORE]),
            "wtm": np.ascontiguousarray(np.asarray(weight_mu)[o0:o0 + O_CORE, :].T),
            "wtr": np.ascontiguousarray(np.asarray(weight_rho)[o0:o0 + O_CORE, :].T),
            "wte": np.ascontiguousarray(np.asarray(weight_eps)[o0:o0 + O_CORE, :].T),
            "bm": np.asarray(bias_mu)[o0:o0 + O_CORE],
            "br": np.asarray(bias_rho)[o0:o0 + O_CORE],
            "be": np.asarray(bias_eps)[o0:o0 + O_CORE],
        })
    return in_maps


def run_device(in_maps, inner_reps=1):
    import jax
    from jax.sharding import NamedSharding, PartitionSpec

    fn, mesh, in_names, out_names, out_avals = get_compiled(inner_reps)
    sh = NamedSharding(mesh, PartitionSpec("core"))
    concat_in = [
        np.concatenate([np.asarray(in_maps[c][nm]) for c in range(8)], axis=0)
        for nm in in_names
    ]
    dev_in = [jax.device_put(a, sh) for a in concat_in]
    dev_z = [
        jax.device_put(np.zeros((8 * a.shape[0], *a.shape[1:]), a.dtype), sh)
        for a in out_avals
    ]
    out_arrs = fn(*dev_in, *dev_z)
    jax.block_until_ready(out_arrs)
    i_out = out_names.index("out")
    outs = np.asarray(out_arrs[i_out]).reshape(8, O_CORE, B_CORE)
    return outs, (fn, dev_in, dev_z)


def assemble(outs):
    full = np.empty((BATCH, OUT_F), dtype=np.float32)
    for c in range(8):
        h, g = divmod(c, 4)
        full[h * B_CORE:(h + 1) * B_CORE, g * O_CORE:(g + 1) * O_CORE] = outs[c].T
    return full


def kernel(**inputs) -> np.ndarray:
    in_maps = shard_inputs(**inputs)
    outs, _ = run_device(in_maps)
    return assemble(outs)


if __name__ == "__main__":
    rng = np.random.default_rng(0)
    ins = {
        "x": rng.standard_normal((BATCH, IN_F), dtype=np.float32),
        "weight_mu": (rng.standard_normal((OUT_F, IN_F), dtype=np.float32)
                      * np.sqrt(2.0 / IN_F)).astype(np.float32),
        "weight_rho": rng.uniform(-5.5, -2.5, (OUT_F, IN_F)).astype(np.float32),
        "bias_mu": np.zeros(OUT_F, dtype=np.float32),
        "bias_rho": rng.uniform(-5.5, -2.5, OUT_F).astype(np.float32),
        "weight_eps": rng.standard_normal((OUT_F, IN_F), dtype=np.float32),
        "bias_eps": rng.standard_normal(OUT_F, dtype=np.float32),
    }
    got = kernel(**ins)
    w = ins["weight_mu"] + (np.log1p(np.exp(ins["weight_rho"].astype(np.float64))) + 1e-8) * ins["weight_eps"]
    b = ins["bias_mu"] + (np.log1p(np.exp(ins["bias_rho"].astype(np.float64))) + 1e-8) * ins["bias_eps"]
    ref = ins["x"].astype(np.float64) @ w.T + b
    rel = np.linalg.norm(got - ref) / np.linalg.norm(ref)
    print("L2 rel err vs fp64 numpy:", rel)


# revision 13
# speedup vs baseline: 3.2296x; 1.3411x over previous
"""BayesianLinear TRN2 kernel: out = x @ (mu + (softplus(rho)+1e-8)*eps).T + bias.

Full shapes: x [4096, 4096], weight_* [4096(out), 4096(in)], bias_* [4096].
Sharding across 8 NeuronCores: 2 batch-halves x 4 out-groups.
  core c: batch rows [ (c//4)*2048 : ... ), out cols [ (c%4)*1024 : ... ).
Per core the kernel computes the TRANSPOSED shard outT [1024(out), 2048(batch)]
= W_g @ x_h.T; the host assemble() transposes back (layout-only).

v2 design (vs v1 which was x-stationary f32r):
- The sampled weight W = mu + softplus(rho)*eps is the matmul STATIONARY
  operand in BF16 — it is produced directly by the DVE add (output cast),
  so no extra conversion pass. bf16 stationary gets the fast weight-load
  path; f32r stationary self-loads 4B columns serially per matmul and was
  the main model-vs-hardware gap in v1.
- x streams as the MOVING operand in f32r via AP bitcast of the f32 DMA
  bytes: no gpsimd conversion pass at all. Moving f32r at free-size 512
  runs 1 row/cycle (full rate).
- Output tiles are [128 out, 512 batch] in PSUM (1 bank), accumulated over
  4 K-chunks per phase, 8 phases; SBUF fp32 accumulator across phases.
- Bias is per-partition in this layout: one tensor_scalar add with a
  [128,1] scalar AP, fused into the phase-0 accumulator write.

Per-core roofline: DMA 92 MB / 360 GBps = 256 us; PE 1024 matmuls x 512
rows x 0.4167 ns = 218 us. Target ~260-280 us.
"""
import numpy as np
from contextlib import ExitStack

import concourse.tile as tile
import concourse.mybir as mybir
from concourse import bacc

P = 128
IN_F = 4096           # contraction (in_features)
BATCH = 4096
OUT_F = 4096
B_CORE = 2048         # batch cols per core (2 halves)
O_CORE = 1024         # out rows per core (4 groups)
N_KC = IN_F // P      # 32 k-chunks of 128
N_PHASES = 8
KC_P = N_KC // N_PHASES   # 4 k-chunks per phase
OT = O_CORE // P      # 8 out-tiles of 128
BG = B_CORE // 512    # 4 batch-groups of 512

F32 = mybir.dt.float32
F32R = mybir.dt.float32r
BF16 = mybir.dt.bfloat16
ACT = mybir.ActivationFunctionType
ALU = mybir.AluOpType

_CACHE = {}


def build_nc(inner_reps=1):
    nc = bacc.Bacc("TRN2", debug=False, num_devices=8)
    xt = nc.dram_tensor("xt", (IN_F, B_CORE), F32, kind="ExternalInput").ap()
    wtm = nc.dram_tensor("wtm", (IN_F, O_CORE), F32, kind="ExternalInput").ap()
    wtr = nc.dram_tensor("wtr", (IN_F, O_CORE), F32, kind="ExternalInput").ap()
    wte = nc.dram_tensor("wte", (IN_F, O_CORE), F32, kind="ExternalInput").ap()
    bm = nc.dram_tensor("bm", (O_CORE,), F32, kind="ExternalInput").ap()
    br = nc.dram_tensor("br", (O_CORE,), F32, kind="ExternalInput").ap()
    be = nc.dram_tensor("be", (O_CORE,), F32, kind="ExternalInput").ap()
    out = nc.dram_tensor("out", (O_CORE, B_CORE), F32, kind="ExternalOutput").ap()

    xt_r = xt.rearrange("(kc p) b -> p kc b", p=P)       # [128, 32, 2048]
    wm_r = wtm.rearrange("(kc p) o -> kc p o", p=P)      # [32, 128, 1024]
    wr_r = wtr.rearrange("(kc p) o -> kc p o", p=P)
    we_r = wte.rearrange("(kc p) o -> kc p o", p=P)
    out_r = out.rearrange("(ot p) b -> ot p b", p=P)     # [8, 128, 2048]
    bm_r = bm.rearrange("(ot p) -> p ot", p=P)           # [128, 8]
    br_r = br.rearrange("(ot p) -> p ot", p=P)
    be_r = be.rearrange("(ot p) -> p ot", p=P)

    with ExitStack() as ctx:
        tc = ctx.enter_context(tile.TileContext(nc))
        wstage = ctx.enter_context(tc.tile_pool(name="ws", bufs=3))
        wpool = ctx.enter_context(tc.tile_pool(name="w", bufs=2))
        xpool = ctx.enter_context(tc.tile_pool(name="x", bufs=2))
        accpool = ctx.enter_context(tc.tile_pool(name="acc", bufs=1))
        bpool = ctx.enter_context(tc.tile_pool(name="bias", bufs=1))
        pspool = ctx.enter_context(tc.tile_pool(name="ps", bufs=2, space="PSUM"))

        acc = accpool.tile([P, OT, B_CORE], F32)        # 64KB/partition
        bias_t = bpool.tile([P, OT], F32, tag="bias")

        def prep_bias():
            tb_r = bpool.tile([P, OT], F32, tag="b_r")
            tb_m = bpool.tile([P, OT], F32, tag="b_m")
            tb_e = bpool.tile([P, OT], F32, tag="b_e")
            nc.scalar.dma_start(tb_r[:], br_r)
            nc.scalar.dma_start(tb_m[:], bm_r)
            nc.scalar.dma_start(tb_e[:], be_r)
            nc.scalar.activation(tb_r[:], tb_r[:], ACT.Exp)
            nc.scalar.activation(tb_r[:], tb_r[:], ACT.Ln, bias=1.0)
            nc.vector.scalar_tensor_tensor(tb_r[:], tb_r[:], 1e-8, tb_e[:],
                                           ALU.add, ALU.mult)
            nc.vector.tensor_add(bias_t[:], tb_r[:], tb_m[:])

        def prep_phase(p):
            kc0 = KC_P * p
            w_p = wpool.tile([P, KC_P, O_CORE], BF16, tag="w", name="w_p")
            for kc in range(KC_P):
                k = kc0 + kc
                t_r = wstage.tile([P, O_CORE], F32, tag="rho")
                t_m = wstage.tile([P, O_CORE], F32, tag="mu")
                t_e = wstage.tile([P, O_CORE], F32, tag="eps")
                nc.sync.dma_start(t_r[:], wr_r[k])
                nc.sync.dma_start(t_m[:], wm_r[k])
                nc.sync.dma_start(t_e[:], we_r[k])
                nc.scalar.activation(t_r[:], t_r[:], ACT.Exp)
                nc.scalar.activation(t_r[:], t_r[:], ACT.Ln, bias=1.0)
                # (sigma+1e-8)*eps; in-place into t_r
                nc.vector.scalar_tensor_tensor(t_r[:], t_r[:], 1e-8, t_e[:],
                                               ALU.add, ALU.mult)
                nc.vector.tensor_add(w_p[:, kc], t_r[:], t_m[:])
            return w_p

        for _rep in range(inner_reps):
            if _rep == 0:
                prep_bias()
            for p in range(N_PHASES):
                kc0 = KC_P * p
                w_p = prep_phase(p)
                # x: HWDGE f32 loads chunked per 512-batch group, cast to bf16
                # on the otherwise-idle Pool engine. (A casting SWDGE DMA would
                # skip the copy, but software descriptor prep is slow on HW.)
                xs = xpool.tile([P, KC_P, B_CORE], BF16, tag="xs", name="xs",
                                bufs=3)
                for bg in range(BG):
                    sl = slice(bg * 512, (bg + 1) * 512)
                    xs32 = xpool.tile([P, KC_P, 512], F32, tag="xs32",
                                      name="xs32", bufs=5)
                    nc.sync.dma_start(xs32[:],
                                        xt_r[:, kc0:kc0 + KC_P, sl])
                    nc.gpsimd.tensor_copy(xs[:, :, sl], xs32[:])
                for ot in range(OT):
                    ps = pspool.tile([P, B_CORE], F32, tag="ps")
                    for kc in range(KC_P):
                        for bg in range(BG):
                            nc.tensor.matmul(
                                ps[:, bg * 512:(bg + 1) * 512],
                                w_p[:, kc, ot * P:(ot + 1) * P],
                                xs[:, kc, bg * 512:(bg + 1) * 512],
                                start=(kc == 0),
                                stop=(kc == KC_P - 1),
                            )
                    a = acc[:, ot, :]
                    if p == 0:
                        nc.vector.tensor_scalar(
                            a, ps[:], bias_t[:, ot:ot + 1], None, ALU.add)
                    else:
                        nc.vector.tensor_add(a, a, ps[:])
                    if p == N_PHASES - 1:
                        nc.scalar.dma_start(out_r[ot], a)
    nc.compile()
    return nc


# ---------------------------------------------------------------------------
# host-side runner (PJRT under axon)
# ---------------------------------------------------------------------------

def _prepare_fn(nc, n_cores=8):
    import jax
    from jax.sharding import Mesh, PartitionSpec
    from jax.experimental.shard_map import shard_map
    from concourse.bass2jax import (
        _bass_exec_p, install_neuronx_cc_hook, partition_id_tensor,
    )

    install_neuronx_cc_hook()
    pname = nc.partition_id_tensor.name if nc.partition_id_tensor else None
    in_names, out_names, out_avals = [], [], []
    for alloc in nc.m.functions[0].allocations:
        if not isinstance(alloc, mybir.MemoryLocationSet):
            continue
        name = alloc.memorylocations[0].name
        if alloc.kind == "ExternalInput":
            if name != pname:
                in_names.append(name)
        elif alloc.kind == "ExternalOutput":
            out_names.append(name)
            out_avals.append(
                jax.core.ShapedArray(tuple(alloc.tensor_shape), mybir.dt.np(alloc.dtype))
            )

    all_in = list(in_names) + list(out_names) + ([pname] if pname else [])

    def _body(*args):
        ops = list(args)
        if pname:
            ops.append(partition_id_tensor())
        return tuple(
            _bass_exec_p.bind(
                *ops,
                out_avals=tuple(out_avals),
                in_names=tuple(all_in),
                out_names=tuple(out_names),
                lowering_input_output_aliases=(),
                sim_require_finite=True,
                sim_require_nnan=True,
                nc=nc,
            )
        )

    devices = jax.devices()[:n_cores]
    mesh = Mesh(np.asarray(devices), ("core",))
    nargs = len(in_names) + len(out_names)
    fn = jax.jit(
        shard_map(
            _body, mesh=mesh,
            in_specs=(PartitionSpec("core"),) * nargs,
            out_specs=(PartitionSpec("core"),) * len(out_names),
            check_rep=False,
        ),
        keep_unused=True,
    )
    return fn, mesh, in_names, out_names, out_avals


def get_compiled(inner_reps=1):
    key = ("fn", inner_reps)
    if key not in _CACHE:
        nc = build_nc(inner_reps)
        _CACHE[key] = _prepare_fn(nc)
    return _CACHE[key]


def shard_inputs(x, weight_mu, weight_rho, bias_mu, bias_rho, weight_eps, bias_eps):
    """Returns in_maps (list of dicts, one per core). Layout-only transforms."""
    xT = np.ascontiguousarray(np.asarray(x).T)          # [in, batch]
    in_maps = []
    for c in range(8):
        h, g = divmod(c, 4)
        o0 = g * O_CORE
        in_maps.append({
            "xt": np.ascontiguousarray(xT[:, h * B_CORE:(h + 1) * B_CORE]),
            "wtm": np.ascontiguousarray(np.asarray(weight_mu)[o0:o0 + O_CORE, :].T),
            "wtr": np.ascontiguousarray(np.asarray(weight_rho)[o0:o0 + O_CORE, :].T),
            "wte": np.ascontiguousarray(np.asarray(weight_eps)[o0:o0 + O_CORE, :].T),
            "bm": np.asarray(bias_mu)[o0:o0 + O_CORE],
            "br": np.asarray(bias_rho)[o0:o0 + O_CORE],
            "be": np.asarray(bias_eps)[o0:o0 + O_CORE],
        })
    return in_maps


def run_device(in_maps, inner_reps=1):
    import jax
    from jax.sharding import NamedSharding, PartitionSpec

    fn, mesh, in_names, out_names, out_avals = get_compiled(inner_reps)
    sh = NamedSharding(mesh, PartitionSpec("core"))
    concat_in = [
        np.concatenate([np.asarray(in_maps[c][nm]) for c in range(8)], axis=0)
        for nm in in_names
    ]
    dev_in = [jax.device_put(a, sh) for a in concat_in]
    dev_z = [
        jax.device_put(np.zeros((8 * a.shape[0], *a.shape[1:]), a.dtype), sh)
        for a in out_avals
    ]
    out_arrs = fn(*dev_in, *dev_z)
    jax.block_until_ready(out_arrs)
    i_out = out_names.index("out")
    outs = np.asarray(out_arrs[i_out]).reshape(8, O_CORE, B_CORE)
    return outs, (fn, dev_in, dev_z)


def assemble(outs):
    full = np.empty((BATCH, OUT_F), dtype=np.float32)
    for c in range(8):
        h, g = divmod(c, 4)
        full[h * B_CORE:(h + 1) * B_CORE, g * O_CORE:(g + 1) * O_CORE] = outs[c].T
    return full


def kernel(**inputs) -> np.ndarray:
    in_maps = shard_inputs(**inputs)
    outs, _ = run_device(in_maps)
    return assemble(outs)


if __name__ == "__main__":
    rng = np.random.default_rng(0)
    ins = {
        "x": rng.standard_normal((BATCH, IN_F), dtype=np.float32),
        "weight_mu": (rng.standard_normal((OUT_F, IN_F), dtype=np.float32)
                      * np.sqrt(2.0 / IN_F)).astype(np.float32),
        "weight_rho": rng.uniform(-5.5, -2.5, (OUT_F, IN_F)).astype(np.float32),
        "bias_mu": np.zeros(OUT_F, dtype=np.float32),
        "bias_rho": rng.uniform(-5.5, -2.5, OUT_F).astype(np.float32),
        "weight_eps": rng.standard_normal((OUT_F, IN_F), dtype=np.float32),
        "bias_eps": rng.standard_normal(OUT_F, dtype=np.float32),
    }
    got = kernel(**ins)
    w = ins["weight_mu"] + (np.log1p(np.exp(ins["weight_rho"].astype(np.float64))) + 1e-8) * ins["weight_eps"]
    b = ins["bias_mu"] + (np.log1p(np.exp(ins["bias_rho"].astype(np.float64))) + 1e-8) * ins["bias_eps"]
    ref = ins["x"].astype(np.float64) @ w.T + b
    rel = np.linalg.norm(got - ref) / np.linalg.norm(ref)
    print("L2 rel err vs fp64 numpy:", rel)
